# revision 8
# baseline (speedup 1.0000x reference)
"""MoE dispatch (DispatchSF) Trainium2 Bass kernel.

Problem: x[8192,1024] f32, hot_mask[8192,8] int32 (multi-hot 0/1),
score[8192,8] f32.  For each expert e: gather tokens with hot_mask[:,e]==1
in token order, scaled by score[:,e], zero-padded to capacity N; plus the
gathered token ids (tags, 0-padded) and per-expert counts.

Sharding: expert-parallel — core e handles expert e (E == n_cores == 8).
Each core receives the full x plus its expert's mask/score column, and
produces out_data[e], out_tags[e], counts[e] independently (no
collectives).

Device algorithm (per core):
  1. ranks = inclusive cumsum of the mask over token order, computed as
     free-dim scan (64 tokens/partition) + cross-partition exclusive scan
     via a triangular matmul.
  2. Every token gets a unique destination slot: routed tokens compact to
     the front in order (rank-1), dropped tokens compact to the back
     (count + drop_rank-1).  Dropped tokens carry zeros (scale=0), so one
     full-permutation scatter writes the scaled rows AND the zero padding
     in a single 32MB pass — no separate memset of the output.
  3. Reads use an indirect gather with out-of-bounds skip for dropped
     tokens (their rows are never read; the stale SBUF content is zeroed
     by the scale multiply), so only ~count rows of x are read.
  4. Tags are scattered through the same permutation (value = mask*t).
"""

import numpy as np

import concourse.bacc as bacc
import concourse.bass as bass
import concourse.mybir as mybir
import concourse.tile as tile
from concourse.bass_utils import run_bass_kernel_spmd

N, D, E = 8192, 1024, 8
P = 128
F = N // P            # 64 tokens per partition, token t = p*F + f
XBUFS = 16            # x-tile pool depth
OOB = 1 << 20         # gather index for dropped tokens (> bounds -> skipped)

f32 = mybir.dt.float32
i32 = mybir.dt.int32
ALU = mybir.AluOpType


def build_program(gather_skip=True):
    nc = bacc.Bacc(
        "TRN2",
        target_bir_lowering=False,
        debug=False,
        enable_asserts=False,
        num_devices=E,
    )

    x = nc.dram_tensor("x", [N, D], f32, kind="ExternalInput")
    mcol = nc.dram_tensor("mask_col", [N], i32, kind="ExternalInput")
    scol = nc.dram_tensor("score_col", [N], f32, kind="ExternalInput")
    out_data = nc.dram_tensor("out_data", [N, D], f32, kind="ExternalOutput")
    out_tags = nc.dram_tensor("out_tags", [N, 1], i32, kind="ExternalOutput")
    out_cnt = nc.dram_tensor("out_count", [1, 1], i32, kind="ExternalOutput")

    with tile.TileContext(nc) as tc:
        with (
            tc.tile_pool(name="route", bufs=1) as rp,
            tc.tile_pool(name="psum", bufs=1, space="PSUM") as pp,
            tc.tile_pool(name="xdata", bufs=XBUFS) as xp,
        ):
            # ---------------- routing computation (tiny) ----------------
            m_i = rp.tile([P, F], i32)
            s_f = rp.tile([P, F], f32)
            nc.sync.dma_start(m_i[:], mcol.ap().rearrange("(p f) -> p f", f=F))
            nc.sync.dma_start(s_f[:], scol.ap().rearrange("(p f) -> p f", f=F))

            m_f = rp.tile([P, F], f32)
            nc.vector.tensor_copy(m_f[:], m_i[:])

            # inclusive prefix sum along each partition's 64 tokens
            pf = rp.tile([P, F], f32)
            nc.vector.tensor_tensor_scan(
                pf[:], m_f[:], m_f[:], 0.0, ALU.add, ALU.bypass
            )
            rowsum = pf[:, F - 1 : F]  # [P,1] per-partition totals

            # lt[k,i] = 1 if k<i  (strictly-lower in (k,i)); ones[k,i] = 1
            lt = rp.tile([P, P], f32)
            nc.gpsimd.memset(lt[:], 1.0)
            # keep 1.0 where i - k > 0 (i = free idx, k = partition idx)
            nc.gpsimd.affine_select(
                out=lt[:], in_=lt[:], compare_op=ALU.is_gt, fill=0.0,
                base=0, pattern=[[1, P]], channel_multiplier=-1,
            )
            ones = rp.tile([P, P], f32)
            nc.gpsimd.memset(ones[:], 1.0)

            offs_ps = pp.tile([P, 1], f32, space="PSUM")
            cnt_ps = pp.tile([P, 1], f32, space="PSUM")
            # offs[p] = sum_{k<p} rowsum[k];  cnt[p] = total count (all p)
            nc.tensor.matmul(offs_ps[:], lt[:], rowsum, start=True, stop=True)
            nc.tensor.matmul(cnt_ps[:], ones[:], rowsum, start=True, stop=True)
            offs = rp.tile([P, 1], f32)
            cnt = rp.tile([P, 1], f32)
            nc.vector.tensor_copy(offs[:], offs_ps[:])
            nc.vector.tensor_copy(cnt[:], cnt_ps[:])

            # ranks = inclusive cumsum over global token order t = p*F + f
            ranks = rp.tile([P, F], f32)
            nc.vector.tensor_scalar_add(ranks[:], pf[:], offs[:, :1])

            # token index t
            t_i = rp.tile([P, F], i32)
            nc.gpsimd.iota(t_i[:], pattern=[[1, F]], base=0, channel_multiplier=F)
            t_f = rp.tile([P, F], f32)
            nc.vector.tensor_copy(t_f[:], t_i[:])

            # dest slot: routed -> ranks-1 (front), dropped -> cnt + t - ranks (back)
            dr = rp.tile([P, F], f32)
            nc.vector.tensor_scalar_add(dr[:], ranks[:], -1.0)
            dd = rp.tile([P, F], f32)
            nc.vector.tensor_sub(dd[:], t_f[:], ranks[:])
            nc.vector.tensor_scalar_add(dd[:], dd[:], cnt[:, :1])
            dest_f = rp.tile([P, F], f32)
            nc.vector.select(dest_f[:], m_i[:], dr[:], dd[:])
            dest_i = rp.tile([P, F], i32)
            nc.vector.tensor_copy(dest_i[:], dest_f[:])

            # row scale (0 for dropped tokens)
            scale = rp.tile([P, F], f32)
            nc.vector.tensor_mul(scale[:], s_f[:], m_f[:])

            # gather index: routed -> t, dropped -> OOB (skipped by bounds check)
            gidx_i = None
            if gather_skip:
                gidx_f = rp.tile([P, F], f32)
                # (m * -OOB + OOB) = OOB for dropped, 0 for routed; then + t
                nc.vector.tensor_scalar(
                    gidx_f[:], m_f[:], -float(OOB), float(OOB), ALU.mult, ALU.add
                )
                nc.vector.tensor_add(gidx_f[:], gidx_f[:], t_f[:])
                gidx_i = rp.tile([P, F], i32)
                nc.vector.tensor_copy(gidx_i[:], gidx_f[:])

            # tags = mask * t, scattered through the same permutation inside
            # the main loop (one index per partition per op on HW; interleaved
            # so they fill GpSimd's wait gaps instead of delaying the pipeline)
            w_f = rp.tile([P, F], f32)
            nc.vector.tensor_mul(w_f[:], m_f[:], t_f[:])
            w_i = rp.tile([P, F], i32)
            nc.vector.tensor_copy(w_i[:], w_f[:])

            # count output
            cnt_i = rp.tile([1, 1], i32)
            nc.vector.tensor_copy(cnt_i[:], cnt[:1, :1])
            nc.sync.dma_start(out_cnt.ap(), cnt_i[:])

            # ---------------- bulk data movement ----------------
            x3 = x.ap().rearrange("(p f) d -> p f d", f=F)
            if gather_skip:
                # zero the pool slots once so stale SBUF NaNs can't survive
                # scale-by-zero on skipped (never-gathered) rows
                for _ in range(XBUFS):
                    zt = xp.tile([P, D], f32, tag="xt")
                    nc.any.memset(zt[:], 0.0)

            for f in range(F):
                xt = xp.tile([P, D], f32, tag="xt")
                if gather_skip:
                    nc.gpsimd.indirect_dma_start(
                        out=xt[:],
                        out_offset=None,
                        in_=x.ap(),
                        in_offset=bass.IndirectOffsetOnAxis(
                            ap=gidx_i[:, f : f + 1], axis=0
                        ),
                        bounds_check=N - 1,
                        oob_is_err=False,
                    )
                else:
                    nc.sync.dma_start(xt[:], x3[:, f, :])
                nc.vector.tensor_scalar_mul(
                    xt[:], xt[:], scale[:, f : f + 1]
                )
                nc.gpsimd.indirect_dma_start(
                    out=out_tags.ap(),
                    out_offset=bass.IndirectOffsetOnAxis(
                        ap=dest_i[:, f : f + 1], axis=0
                    ),
                    in_=w_i[:, f : f + 1],
                    in_offset=None,
                )
                nc.gpsimd.indirect_dma_start(
                    out=out_data.ap(),
                    out_offset=bass.IndirectOffsetOnAxis(
                        ap=dest_i[:, f : f + 1], axis=0
                    ),
                    in_=xt[:],
                    in_offset=None,
                )

    nc.compile()
    return nc


_nc = None


def _get_program():
    global _nc
    if _nc is None:
        _nc = build_program()
    return _nc


def make_in_maps(x, hot_mask, score):
    x = np.ascontiguousarray(x, dtype=np.float32)
    return [
        {
            "x": x,
            "mask_col": np.ascontiguousarray(hot_mask[:, e], dtype=np.int32),
            "score_col": np.ascontiguousarray(score[:, e], dtype=np.float32),
        }
        for e in range(E)
    ]


def run(x, hot_mask, score, trace=False):
    """Returns ((out_data, out_tags, counts), BassKernelResults)."""
    nc = _get_program()
    res = run_bass_kernel_spmd(
        nc, make_in_maps(x, hot_mask, score), core_ids=list(range(E)), trace=trace
    )
    out_data = np.stack([r["out_data"] for r in res.results])
    out_tags = np.stack([r["out_tags"].reshape(N) for r in res.results])
    counts = np.array([r["out_count"].reshape(()) for r in res.results], dtype=np.int32)
    return (out_data, out_tags, counts), res


def kernel(x, hot_mask, score):
    return run(x, hot_mask, score)[0]


# revision 9
# speedup vs baseline: 1.0811x; 1.0811x over previous
"""MoE dispatch (DispatchSF) Trainium2 Bass kernel.

Problem: x[8192,1024] f32, hot_mask[8192,8] int32 (multi-hot 0/1),
score[8192,8] f32.  For each expert e: gather tokens with hot_mask[:,e]==1
in token order, scaled by score[:,e], zero-padded to capacity N; plus the
gathered token ids (tags, 0-padded) and per-expert counts.

Sharding: expert-parallel — core e handles expert e (E == n_cores == 8).
Each core receives the full x plus its expert's mask/score column, and
produces out_data[e], out_tags[e], counts[e] independently (no
collectives).

Device algorithm (per core):
  1. ranks = inclusive cumsum of the mask over token order, computed as
     free-dim scan (64 tokens/partition) + cross-partition exclusive scan
     via a triangular matmul.
  2. Every token gets a unique destination slot: routed tokens compact to
     the front in order (rank-1), dropped tokens compact to the back
     (count + drop_rank-1).  Dropped tokens carry zeros (scale=0), so one
     full-permutation scatter writes the scaled rows AND the zero padding
     in a single 32MB pass — no separate memset of the output.
  3. Reads use an indirect gather with out-of-bounds skip for dropped
     tokens (their rows are never read; the stale SBUF content is zeroed
     by the scale multiply), so only ~count rows of x are read.
  4. Tags are scattered through the same permutation (value = mask*t).
"""

import numpy as np

import concourse.bacc as bacc
import concourse.bass as bass
import concourse.mybir as mybir
import concourse.tile as tile
from concourse.bass_utils import run_bass_kernel_spmd

N, D, E = 8192, 1024, 8
P = 128
F = N // P            # 64 tokens per partition, token t = p*F + f
XBUFS = 16            # x-tile pool depth
OOB = 1 << 20         # gather index for dropped tokens (> bounds -> skipped)

f32 = mybir.dt.float32
i32 = mybir.dt.int32
ALU = mybir.AluOpType


def build_program(gather_skip=True):
    nc = bacc.Bacc(
        "TRN2",
        target_bir_lowering=False,
        debug=False,
        enable_asserts=False,
        num_devices=E,
    )

    x = nc.dram_tensor("x", [N, D], f32, kind="ExternalInput")
    mcol = nc.dram_tensor("mask_col", [N], i32, kind="ExternalInput")
    scol = nc.dram_tensor("score_col", [N], f32, kind="ExternalInput")
    out_data = nc.dram_tensor("out_data", [N, D], f32, kind="ExternalOutput")
    out_tags = nc.dram_tensor("out_tags", [N, 1], i32, kind="ExternalOutput")
    out_cnt = nc.dram_tensor("out_count", [1, 1], i32, kind="ExternalOutput")

    with tile.TileContext(nc) as tc:
        with (
            tc.tile_pool(name="route", bufs=1) as rp,
            tc.tile_pool(name="psum", bufs=1, space="PSUM") as pp,
            tc.tile_pool(name="xdata", bufs=XBUFS) as xp,
        ):
            # ---------------- routing computation (tiny) ----------------
            m_i = rp.tile([P, F], i32)
            s_f = rp.tile([P, F], f32)
            nc.sync.dma_start(m_i[:], mcol.ap().rearrange("(p f) -> p f", f=F))
            nc.sync.dma_start(s_f[:], scol.ap().rearrange("(p f) -> p f", f=F))

            m_f = rp.tile([P, F], f32)
            nc.vector.tensor_copy(m_f[:], m_i[:])

            # inclusive prefix sum along each partition's 64 tokens
            pf = rp.tile([P, F], f32)
            nc.vector.tensor_tensor_scan(
                pf[:], m_f[:], m_f[:], 0.0, ALU.add, ALU.bypass
            )
            rowsum = pf[:, F - 1 : F]  # [P,1] per-partition totals

            # lt[k,i] = 1 if k<i  (strictly-lower in (k,i)); ones[k,i] = 1
            lt = rp.tile([P, P], f32)
            nc.gpsimd.memset(lt[:], 1.0)
            # keep 1.0 where i - k > 0 (i = free idx, k = partition idx)
            nc.gpsimd.affine_select(
                out=lt[:], in_=lt[:], compare_op=ALU.is_gt, fill=0.0,
                base=0, pattern=[[1, P]], channel_multiplier=-1,
            )
            ones = rp.tile([P, P], f32)
            nc.gpsimd.memset(ones[:], 1.0)

            offs_ps = pp.tile([P, 1], f32, space="PSUM")
            cnt_ps = pp.tile([P, 1], f32, space="PSUM")
            # offs[p] = sum_{k<p} rowsum[k];  cnt[p] = total count (all p)
            nc.tensor.matmul(offs_ps[:], lt[:], rowsum, start=True, stop=True)
            nc.tensor.matmul(cnt_ps[:], ones[:], rowsum, start=True, stop=True)
            offs = rp.tile([P, 1], f32)
            cnt = rp.tile([P, 1], f32)
            nc.vector.tensor_copy(offs[:], offs_ps[:])
            nc.vector.tensor_copy(cnt[:], cnt_ps[:])

            # ranks = inclusive cumsum over global token order t = p*F + f
            ranks = rp.tile([P, F], f32)
            nc.vector.tensor_scalar_add(ranks[:], pf[:], offs[:, :1])

            # token index t
            t_i = rp.tile([P, F], i32)
            nc.gpsimd.iota(t_i[:], pattern=[[1, F]], base=0, channel_multiplier=F)
            t_f = rp.tile([P, F], f32)
            nc.vector.tensor_copy(t_f[:], t_i[:])

            # dest slot: routed -> ranks-1 (front), dropped -> cnt + t - ranks (back)
            dr = rp.tile([P, F], f32)
            nc.vector.tensor_scalar_add(dr[:], ranks[:], -1.0)
            dd = rp.tile([P, F], f32)
            nc.vector.tensor_sub(dd[:], t_f[:], ranks[:])
            nc.vector.tensor_scalar_add(dd[:], dd[:], cnt[:, :1])
            dest_f = rp.tile([P, F], f32)
            nc.vector.select(dest_f[:], m_i[:], dr[:], dd[:])
            dest_i = rp.tile([P, F], i32)
            nc.vector.tensor_copy(dest_i[:], dest_f[:])

            # row scale (0 for dropped tokens)
            scale = rp.tile([P, F], f32)
            nc.vector.tensor_mul(scale[:], s_f[:], m_f[:])

            # gather index: routed -> t, dropped -> OOB (skipped by bounds check)
            gidx_i = None
            if gather_skip:
                gidx_f = rp.tile([P, F], f32)
                # (m * -OOB + OOB) = OOB for dropped, 0 for routed; then + t
                nc.vector.tensor_scalar(
                    gidx_f[:], m_f[:], -float(OOB), float(OOB), ALU.mult, ALU.add
                )
                nc.vector.tensor_add(gidx_f[:], gidx_f[:], t_f[:])
                gidx_i = rp.tile([P, F], i32)
                nc.vector.tensor_copy(gidx_i[:], gidx_f[:])

            # tags = mask * t, scattered through the same permutation inside
            # the main loop (one index per partition per op on HW; interleaved
            # so they fill GpSimd's wait gaps instead of delaying the pipeline)
            w_f = rp.tile([P, F], f32)
            nc.vector.tensor_mul(w_f[:], m_f[:], t_f[:])
            w_i = rp.tile([P, F], i32)
            nc.vector.tensor_copy(w_i[:], w_f[:])

            # count output
            cnt_i = rp.tile([1, 1], i32)
            nc.vector.tensor_copy(cnt_i[:], cnt[:1, :1])
            nc.sync.dma_start(out_cnt.ap(), cnt_i[:])

            # ---------------- bulk data movement ----------------
            x3 = x.ap().rearrange("(p f) d -> p f d", f=F)
            if gather_skip:
                # zero the pool slots once so stale SBUF NaNs can't survive
                # scale-by-zero on skipped (never-gathered) rows
                for _ in range(XBUFS):
                    zt = xp.tile([P, D], f32, tag="xt")
                    nc.any.memset(zt[:], 0.0)

            for f in range(F):
                xt = xp.tile([P, D], f32, tag="xt")
                if gather_skip:
                    nc.gpsimd.indirect_dma_start(
                        out=xt[:],
                        out_offset=None,
                        in_=x.ap(),
                        in_offset=bass.IndirectOffsetOnAxis(
                            ap=gidx_i[:, f : f + 1], axis=0
                        ),
                        bounds_check=N - 1,
                        oob_is_err=False,
                    )
                else:
                    nc.sync.dma_start(xt[:], x3[:, f, :])
                nc.vector.tensor_scalar_mul(
                    xt[:], xt[:], scale[:, f : f + 1]
                )
                nc.gpsimd.indirect_dma_start(
                    out=out_data.ap(),
                    out_offset=bass.IndirectOffsetOnAxis(
                        ap=dest_i[:, f : f + 1], axis=0
                    ),
                    in_=xt[:],
                    in_offset=None,
                )
            # tag scatters after the bulk loop so they don't delay it
            for f in range(F):
                nc.gpsimd.indirect_dma_start(
                    out=out_tags.ap(),
                    out_offset=bass.IndirectOffsetOnAxis(
                        ap=dest_i[:, f : f + 1], axis=0
                    ),
                    in_=w_i[:, f : f + 1],
                    in_offset=None,
                )

    nc.compile()
    return nc


_nc = None


def _get_program():
    global _nc
    if _nc is None:
        _nc = build_program()
    return _nc


def make_in_maps(x, hot_mask, score):
    x = np.ascontiguousarray(x, dtype=np.float32)
    return [
        {
            "x": x,
            "mask_col": np.ascontiguousarray(hot_mask[:, e], dtype=np.int32),
            "score_col": np.ascontiguousarray(score[:, e], dtype=np.float32),
        }
        for e in range(E)
    ]


def run(x, hot_mask, score, trace=False):
    """Returns ((out_data, out_tags, counts), BassKernelResults)."""
    nc = _get_program()
    res = run_bass_kernel_spmd(
        nc, make_in_maps(x, hot_mask, score), core_ids=list(range(E)), trace=trace
    )
    out_data = np.stack([r["out_data"] for r in res.results])
    out_tags = np.stack([r["out_tags"].reshape(N) for r in res.results])
    counts = np.array([r["out_count"].reshape(()) for r in res.results], dtype=np.int32)
    return (out_data, out_tags, counts), res


def kernel(x, hot_mask, score):
    return run(x, hot_mask, score)[0]


# revision 12
# speedup vs baseline: 1.1298x; 1.0450x over previous
"""MoE dispatch (DispatchSF) Trainium2 Bass kernel.

Problem: x[8192,1024] f32, hot_mask[8192,8] int32 (multi-hot 0/1),
score[8192,8] f32.  For each expert e: gather tokens with hot_mask[:,e]==1
in token order, scaled by score[:,e], zero-padded to capacity N; plus the
gathered token ids (tags, 0-padded) and per-expert counts.

Sharding: expert-parallel — core e handles expert e (E == n_cores == 8).
Each core receives the full x plus its expert's mask/score column, and
produces out_data[e], out_tags[e], counts[e] independently (no
collectives).

Device algorithm (per core):
  1. ranks = inclusive cumsum of the mask over token order, computed as
     free-dim scan (64 tokens/partition) + cross-partition exclusive scan
     via a triangular matmul.
  2. Every token gets a unique destination slot: routed tokens compact to
     the front in order (rank-1), dropped tokens compact to the back
     (count + drop_rank-1).  Dropped tokens carry zeros (scale=0), so one
     full-permutation scatter writes the scaled rows AND the zero padding
     in a single 32MB pass — no separate memset of the output.
  3. Reads use an indirect gather with out-of-bounds skip for dropped
     tokens (their rows are never read; the stale SBUF content is zeroed
     by the scale multiply), so only ~count rows of x are read.
  4. Tags are scattered through the same permutation (value = mask*t).
"""

import numpy as np

import concourse.bacc as bacc
import concourse.bass as bass
import concourse.mybir as mybir
import concourse.tile as tile
from concourse.bass_utils import run_bass_kernel_spmd

N, D, E = 8192, 1024, 8
P = 128
F = N // P            # 64 tokens per partition, token t = p*F + f
XBUFS = 8             # x-tile pool depth
OOB = 1 << 20         # gather index for dropped tokens (> bounds -> skipped)

f32 = mybir.dt.float32
i32 = mybir.dt.int32
ALU = mybir.AluOpType


def build_program(gather_skip=True):
    nc = bacc.Bacc(
        "TRN2",
        target_bir_lowering=False,
        debug=False,
        enable_asserts=False,
        num_devices=E,
    )

    x = nc.dram_tensor("x", [N, D], f32, kind="ExternalInput")
    mcol = nc.dram_tensor("mask_col", [N], i32, kind="ExternalInput")
    scol = nc.dram_tensor("score_col", [N], f32, kind="ExternalInput")
    out_data = nc.dram_tensor("out_data", [N, D], f32, kind="ExternalOutput")
    out_tags = nc.dram_tensor("out_tags", [N, 1], i32, kind="ExternalOutput")
    out_cnt = nc.dram_tensor("out_count", [1, 1], i32, kind="ExternalOutput")

    with tile.TileContext(nc) as tc:
        with (
            tc.tile_pool(name="route", bufs=1) as rp,
            tc.tile_pool(name="psum", bufs=1, space="PSUM") as pp,
            tc.tile_pool(name="xdata", bufs=XBUFS) as xp,
        ):
            # ---------------- routing computation (tiny) ----------------
            m_i = rp.tile([P, F], i32)
            s_f = rp.tile([P, F], f32)
            nc.sync.dma_start(m_i[:], mcol.ap().rearrange("(p f) -> p f", f=F))
            nc.sync.dma_start(s_f[:], scol.ap().rearrange("(p f) -> p f", f=F))

            m_f = rp.tile([P, F], f32)
            nc.vector.tensor_copy(m_f[:], m_i[:])

            # inclusive prefix sum along each partition's 64 tokens
            pf = rp.tile([P, F], f32)
            nc.vector.tensor_tensor_scan(
                pf[:], m_f[:], m_f[:], 0.0, ALU.add, ALU.bypass
            )
            rowsum = pf[:, F - 1 : F]  # [P,1] per-partition totals

            # lt[k,i] = 1 if k<i  (strictly-lower in (k,i)); ones[k,i] = 1
            lt = rp.tile([P, P], f32)
            nc.gpsimd.memset(lt[:], 1.0)
            # keep 1.0 where i - k > 0 (i = free idx, k = partition idx)
            nc.gpsimd.affine_select(
                out=lt[:], in_=lt[:], compare_op=ALU.is_gt, fill=0.0,
                base=0, pattern=[[1, P]], channel_multiplier=-1,
            )
            ones = rp.tile([P, P], f32)
            nc.gpsimd.memset(ones[:], 1.0)

            offs_ps = pp.tile([P, 1], f32, space="PSUM")
            cnt_ps = pp.tile([P, 1], f32, space="PSUM")
            # offs[p] = sum_{k<p} rowsum[k];  cnt[p] = total count (all p)
            nc.tensor.matmul(offs_ps[:], lt[:], rowsum, start=True, stop=True)
            nc.tensor.matmul(cnt_ps[:], ones[:], rowsum, start=True, stop=True)
            offs = rp.tile([P, 1], f32)
            cnt = rp.tile([P, 1], f32)
            nc.vector.tensor_copy(offs[:], offs_ps[:])
            nc.vector.tensor_copy(cnt[:], cnt_ps[:])

            # ranks = inclusive cumsum over global token order t = p*F + f
            ranks = rp.tile([P, F], f32)
            nc.vector.tensor_scalar_add(ranks[:], pf[:], offs[:, :1])

            # token index t
            t_i = rp.tile([P, F], i32)
            nc.gpsimd.iota(t_i[:], pattern=[[1, F]], base=0, channel_multiplier=F)
            t_f = rp.tile([P, F], f32)
            nc.vector.tensor_copy(t_f[:], t_i[:])

            # dest slot: routed -> ranks-1 (front), dropped -> cnt + t - ranks (back)
            dr = rp.tile([P, F], f32)
            nc.vector.tensor_scalar_add(dr[:], ranks[:], -1.0)
            dd = rp.tile([P, F], f32)
            nc.vector.tensor_sub(dd[:], t_f[:], ranks[:])
            nc.vector.tensor_scalar_add(dd[:], dd[:], cnt[:, :1])
            dest_f = rp.tile([P, F], f32)
            nc.vector.select(dest_f[:], m_i[:], dr[:], dd[:])
            dest_i = rp.tile([P, F], i32)
            nc.vector.tensor_copy(dest_i[:], dest_f[:])

            # row scale (0 for dropped tokens)
            scale = rp.tile([P, F], f32)
            nc.vector.tensor_mul(scale[:], s_f[:], m_f[:])

            # gather index: routed -> t, dropped -> OOB (skipped by bounds check)
            gidx_i = None
            if gather_skip:
                gidx_f = rp.tile([P, F], f32)
                # (m * -OOB + OOB) = OOB for dropped, 0 for routed; then + t
                nc.vector.tensor_scalar(
                    gidx_f[:], m_f[:], -float(OOB), float(OOB), ALU.mult, ALU.add
                )
                nc.vector.tensor_add(gidx_f[:], gidx_f[:], t_f[:])
                gidx_i = rp.tile([P, F], i32)
                nc.vector.tensor_copy(gidx_i[:], gidx_f[:])

            # tags = mask * t, scattered through the same permutation
            # (hardware indirect DMA handles one index per partition per op)
            w_f = rp.tile([P, F], f32)
            nc.vector.tensor_mul(w_f[:], m_f[:], t_f[:])
            w_i = rp.tile([P, F], i32)
            nc.vector.tensor_copy(w_i[:], w_f[:])
            for f in range(F):
                nc.gpsimd.indirect_dma_start(
                    out=out_tags.ap(),
                    out_offset=bass.IndirectOffsetOnAxis(
                        ap=dest_i[:, f : f + 1], axis=0
                    ),
                    in_=w_i[:, f : f + 1],
                    in_offset=None,
                )

            # count output
            cnt_i = rp.tile([1, 1], i32)
            nc.vector.tensor_copy(cnt_i[:], cnt[:1, :1])
            nc.sync.dma_start(out_cnt.ap(), cnt_i[:])

            # ---------------- bulk data movement ----------------
            x3 = x.ap().rearrange("(p f) d -> p f d", f=F)
            if gather_skip:
                # zero the pool slots once so stale SBUF NaNs can't survive
                # scale-by-zero on skipped (never-gathered) rows
                for _ in range(XBUFS):
                    zt = xp.tile([P, D], f32, tag="xt")
                    nc.any.memset(zt[:], 0.0)

            for f in range(F):
                xt = xp.tile([P, D], f32, tag="xt")
                if gather_skip:
                    nc.gpsimd.indirect_dma_start(
                        out=xt[:],
                        out_offset=None,
                        in_=x.ap(),
                        in_offset=bass.IndirectOffsetOnAxis(
                            ap=gidx_i[:, f : f + 1], axis=0
                        ),
                        bounds_check=N - 1,
                        oob_is_err=False,
                    )
                else:
                    nc.sync.dma_start(xt[:], x3[:, f, :])
                nc.vector.tensor_scalar_mul(
                    xt[:], xt[:], scale[:, f : f + 1]
                )
                nc.gpsimd.indirect_dma_start(
                    out=out_data.ap(),
                    out_offset=bass.IndirectOffsetOnAxis(
                        ap=dest_i[:, f : f + 1], axis=0
                    ),
                    in_=xt[:],
                    in_offset=None,
                )

    nc.compile()
    return nc


_nc = None


def _get_program():
    global _nc
    if _nc is None:
        _nc = build_program()
    return _nc


def make_in_maps(x, hot_mask, score):
    x = np.ascontiguousarray(x, dtype=np.float32)
    return [
        {
            "x": x,
            "mask_col": np.ascontiguousarray(hot_mask[:, e], dtype=np.int32),
            "score_col": np.ascontiguousarray(score[:, e], dtype=np.float32),
        }
        for e in range(E)
    ]


def run(x, hot_mask, score, trace=False):
    """Returns ((out_data, out_tags, counts), BassKernelResults)."""
    nc = _get_program()
    res = run_bass_kernel_spmd(
        nc, make_in_maps(x, hot_mask, score), core_ids=list(range(E)), trace=trace
    )
    out_data = np.stack([r["out_data"] for r in res.results])
    out_tags = np.stack([r["out_tags"].reshape(N) for r in res.results])
    counts = np.array([r["out_count"].reshape(()) for r in res.results], dtype=np.int32)
    return (out_data, out_tags, counts), res


def kernel(x, hot_mask, score):
    return run(x, hot_mask, score)[0]


# revision 15
# speedup vs baseline: 1.1973x; 1.0598x over previous
"""MoE dispatch (DispatchSF) Trainium2 Bass kernel.

Problem: x[8192,1024] f32, hot_mask[8192,8] int32 (multi-hot 0/1),
score[8192,8] f32.  For each expert e: gather tokens with hot_mask[:,e]==1
in token order, scaled by score[:,e], zero-padded to capacity N; plus the
gathered token ids (tags, 0-padded) and per-expert counts.

Sharding: expert-parallel — core e handles expert e (E == n_cores == 8).
Each core receives the full x plus its expert's mask/score column, and
produces out_data[e], out_tags[e], counts[e] independently (no
collectives).

Device algorithm (per core):
  1. ranks = inclusive cumsum of the mask over token order, computed as
     free-dim scan (64 tokens/partition) + cross-partition exclusive scan
     via a triangular matmul.
  2. Every token gets a unique destination slot: routed tokens compact to
     the front in order (rank-1), dropped tokens compact to the back
     (count + drop_rank-1).  Dropped tokens carry zeros (scale=0), so one
     full-permutation scatter writes the scaled rows AND the zero padding
     in a single 32MB pass — no separate memset of the output.
  3. Reads use an indirect gather with out-of-bounds skip for dropped
     tokens (their rows are never read; the stale SBUF content is zeroed
     by the scale multiply), so only ~count rows of x are read.
  4. Tags are scattered through the same permutation (value = mask*t).
"""

import numpy as np

import concourse.bacc as bacc
import concourse.bass as bass
import concourse.mybir as mybir
import concourse.tile as tile
from concourse.bass_utils import run_bass_kernel_spmd

N, D, E = 8192, 1024, 8
P = 128
F = N // P            # 64 tokens per partition, token t = p*F + f
XBUFS = 8             # x-tile pool depth
OOB = 1 << 20         # gather index for dropped tokens (> bounds -> skipped)

f32 = mybir.dt.float32
i32 = mybir.dt.int32
ALU = mybir.AluOpType


def build_program(gather_skip=True, gather_pair=False):
    nc = bacc.Bacc(
        "TRN2",
        target_bir_lowering=False,
        debug=False,
        enable_asserts=False,
        num_devices=E,
    )

    x = nc.dram_tensor("x", [N, D], f32, kind="ExternalInput")
    mcol = nc.dram_tensor("mask_col", [N], i32, kind="ExternalInput")
    scol = nc.dram_tensor("score_col", [N], f32, kind="ExternalInput")
    out_data = nc.dram_tensor("out_data", [N, D], f32, kind="ExternalOutput")
    out_tags = nc.dram_tensor("out_tags", [N, 1], i32, kind="ExternalOutput")
    out_cnt = nc.dram_tensor("out_count", [1, 1], i32, kind="ExternalOutput")

    with tile.TileContext(nc) as tc:
        with (
            tc.tile_pool(name="route", bufs=1) as rp,
            tc.tile_pool(name="psum", bufs=1, space="PSUM") as pp,
            tc.tile_pool(name="xdata", bufs=XBUFS) as xp,
        ):
            # ---------------- routing computation (tiny) ----------------
            m_i = rp.tile([P, F], i32)
            s_f = rp.tile([P, F], f32)
            nc.sync.dma_start(m_i[:], mcol.ap().rearrange("(p f) -> p f", f=F))
            nc.sync.dma_start(s_f[:], scol.ap().rearrange("(p f) -> p f", f=F))

            m_f = rp.tile([P, F], f32)
            nc.vector.tensor_copy(m_f[:], m_i[:])

            # inclusive prefix sum along each partition's 64 tokens
            pf = rp.tile([P, F], f32)
            nc.vector.tensor_tensor_scan(
                pf[:], m_f[:], m_f[:], 0.0, ALU.add, ALU.bypass
            )
            rowsum = pf[:, F - 1 : F]  # [P,1] per-partition totals

            # lt[k,i] = 1 if k<i  (strictly-lower in (k,i)); ones[k,i] = 1
            lt = rp.tile([P, P], f32)
            nc.gpsimd.memset(lt[:], 1.0)
            # keep 1.0 where i - k > 0 (i = free idx, k = partition idx)
            nc.gpsimd.affine_select(
                out=lt[:], in_=lt[:], compare_op=ALU.is_gt, fill=0.0,
                base=0, pattern=[[1, P]], channel_multiplier=-1,
            )
            ones = rp.tile([P, P], f32)
            nc.gpsimd.memset(ones[:], 1.0)

            offs_ps = pp.tile([P, 1], f32, space="PSUM")
            cnt_ps = pp.tile([P, 1], f32, space="PSUM")
            # offs[p] = sum_{k<p} rowsum[k];  cnt[p] = total count (all p)
            nc.tensor.matmul(offs_ps[:], lt[:], rowsum, start=True, stop=True)
            nc.tensor.matmul(cnt_ps[:], ones[:], rowsum, start=True, stop=True)
            offs = rp.tile([P, 1], f32)
            cnt = rp.tile([P, 1], f32)
            nc.vector.tensor_copy(offs[:], offs_ps[:])
            nc.vector.tensor_copy(cnt[:], cnt_ps[:])

            # ranks = inclusive cumsum over global token order t = p*F + f
            ranks = rp.tile([P, F], f32)
            nc.vector.tensor_scalar_add(ranks[:], pf[:], offs[:, :1])

            # token index t
            t_i = rp.tile([P, F], i32)
            nc.gpsimd.iota(t_i[:], pattern=[[1, F]], base=0, channel_multiplier=F)
            t_f = rp.tile([P, F], f32)
            nc.vector.tensor_copy(t_f[:], t_i[:])

            # dest slot: routed -> ranks-1 (front), dropped -> cnt + t - ranks (back)
            dr = rp.tile([P, F], f32)
            nc.vector.tensor_scalar_add(dr[:], ranks[:], -1.0)
            dd = rp.tile([P, F], f32)
            nc.vector.tensor_sub(dd[:], t_f[:], ranks[:])
            nc.vector.tensor_scalar_add(dd[:], dd[:], cnt[:, :1])
            dest_f = rp.tile([P, F], f32)
            nc.vector.select(dest_f[:], m_i[:], dr[:], dd[:])
            dest_i = rp.tile([P, F], i32)
            nc.vector.tensor_copy(dest_i[:], dest_f[:])

            # row scale (0 for dropped tokens)
            scale = rp.tile([P, F], f32)
            nc.vector.tensor_mul(scale[:], s_f[:], m_f[:])

            # pair-gather indices: one descriptor reads rows (2j, 2j+1) of x;
            # skip only when BOTH tokens of the pair are dropped
            gidxp_i = None
            if gather_pair:
                m2 = m_f[:].rearrange("p (j two) -> p j two", two=2)
                t2 = t_f[:].rearrange("p (j two) -> p j two", two=2)
                m_or = rp.tile([P, F // 2], f32)
                nc.vector.tensor_tensor(
                    out=m_or[:].unsqueeze(-1), in0=m2[:, :, 0:1],
                    in1=m2[:, :, 1:2], op=ALU.max,
                )
                gp_f = rp.tile([P, F // 2], f32)
                nc.vector.tensor_scalar(
                    gp_f[:], m_or[:], -float(OOB), float(OOB), ALU.mult, ALU.add
                )
                nc.vector.tensor_tensor(
                    out=gp_f[:].unsqueeze(-1), in0=gp_f[:].unsqueeze(-1),
                    in1=t2[:, :, 0:1], op=ALU.add,
                )
                gidxp_i = rp.tile([P, F // 2], i32)
                nc.vector.tensor_copy(gidxp_i[:], gp_f[:])

            # gather index: routed -> t, dropped -> OOB (skipped by bounds check)
            gidx_i = None
            if gather_skip and not gather_pair:
                gidx_f = rp.tile([P, F], f32)
                # (m * -OOB + OOB) = OOB for dropped, 0 for routed; then + t
                nc.vector.tensor_scalar(
                    gidx_f[:], m_f[:], -float(OOB), float(OOB), ALU.mult, ALU.add
                )
                nc.vector.tensor_add(gidx_f[:], gidx_f[:], t_f[:])
                gidx_i = rp.tile([P, F], i32)
                nc.vector.tensor_copy(gidx_i[:], gidx_f[:])

            # tags = mask * t, scattered through the same permutation
            # (hardware indirect DMA handles one index per partition per op)
            w_f = rp.tile([P, F], f32)
            nc.vector.tensor_mul(w_f[:], m_f[:], t_f[:])
            w_i = rp.tile([P, F], i32)
            nc.vector.tensor_copy(w_i[:], w_f[:])
            for f in range(F):
                nc.gpsimd.indirect_dma_start(
                    out=out_tags.ap(),
                    out_offset=bass.IndirectOffsetOnAxis(
                        ap=dest_i[:, f : f + 1], axis=0
                    ),
                    in_=w_i[:, f : f + 1],
                    in_offset=None,
                )

            # count output
            cnt_i = rp.tile([1, 1], i32)
            nc.vector.tensor_copy(cnt_i[:], cnt[:1, :1])
            nc.sync.dma_start(out_cnt.ap(), cnt_i[:])

            # ---------------- bulk data movement ----------------
            x3 = x.ap().rearrange("(p f) d -> p f d", f=F)
            if gather_skip:
                # zero the pool slots once so stale SBUF NaNs can't survive
                # scale-by-zero on skipped (never-gathered) rows
                for _ in range(XBUFS):
                    zt = xp.tile([P, D], f32, tag="xt")
                    nc.any.memset(zt[:], 0.0)

            if gather_pair:
                for j in range(F // 2):
                    xt = xp.tile([P, 2, D], f32, tag="xt")
                    nc.gpsimd.indirect_dma_start(
                        out=xt[:],
                        out_offset=None,
                        in_=x.ap(),
                        in_offset=bass.IndirectOffsetOnAxis(
                            ap=gidxp_i[:, j : j + 1], axis=0
                        ),
                        bounds_check=N - 1,
                        oob_is_err=False,
                    )
                    for h in range(2):
                        f = 2 * j + h
                        nc.vector.tensor_scalar_mul(
                            xt[:, h : h + 1, :], xt[:, h : h + 1, :],
                            scale[:, f : f + 1],
                        )
                        nc.gpsimd.indirect_dma_start(
                            out=out_data.ap(),
                            out_offset=bass.IndirectOffsetOnAxis(
                                ap=dest_i[:, f : f + 1], axis=0
                            ),
                            in_=xt[:, h : h + 1, :],
                            in_offset=None,
                        )
            else:
                for f in range(F):
                    xt = xp.tile([P, D], f32, tag="xt")
                    if gather_skip:
                        nc.gpsimd.indirect_dma_start(
                            out=xt[:],
                            out_offset=None,
                            in_=x.ap(),
                            in_offset=bass.IndirectOffsetOnAxis(
                                ap=gidx_i[:, f : f + 1], axis=0
                            ),
                            bounds_check=N - 1,
                            oob_is_err=False,
                        )
                    else:
                        nc.sync.dma_start(xt[:], x3[:, f, :])
                    nc.vector.tensor_scalar_mul(
                        xt[:], xt[:], scale[:, f : f + 1]
                    )
                    nc.gpsimd.indirect_dma_start(
                        out=out_data.ap(),
                        out_offset=bass.IndirectOffsetOnAxis(
                            ap=dest_i[:, f : f + 1], axis=0
                        ),
                        in_=xt[:],
                        in_offset=None,
                    )

    nc.compile()
    return nc


_nc = None


def _get_program():
    global _nc
    if _nc is None:
        _nc = build_program()
    return _nc


def make_in_maps(x, hot_mask, score):
    x = np.ascontiguousarray(x, dtype=np.float32)
    return [
        {
            "x": x,
            "mask_col": np.ascontiguousarray(hot_mask[:, e], dtype=np.int32),
            "score_col": np.ascontiguousarray(score[:, e], dtype=np.float32),
        }
        for e in range(E)
    ]


def run(x, hot_mask, score, trace=False):
    """Returns ((out_data, out_tags, counts), BassKernelResults)."""
    nc = _get_program()
    res = run_bass_kernel_spmd(
        nc, make_in_maps(x, hot_mask, score), core_ids=list(range(E)), trace=trace
    )
    out_data = np.stack([r["out_data"] for r in res.results])
    out_tags = np.stack([r["out_tags"].reshape(N) for r in res.results])
    counts = np.array([r["out_count"].reshape(()) for r in res.results], dtype=np.int32)
    return (out_data, out_tags, counts), res


def kernel(x, hot_mask, score):
    return run(x, hot_mask, score)[0]
